# revision 1
# baseline (speedup 1.0000x reference)
"""Multi-head attention (B=4, S=2048, D=768, H=12) on 8 trn2 NeuronCores.

Sharding: core c handles batch b = c//2 and head-half hh = c%2 (6 heads,
384 features). Each core computes a partial output [2048, 768] (its 6 heads'
contribution through the output projection, un-biased); the host sums the
two partials per batch and adds OW_b plus the V-bias constant
(softmax rows sum to 1, so the V bias contributes OW_w @ VW_b per token).

On-chip dataflow (all matmuls bf16 with fp32 PSUM accumulation):
  x/w loaded fp32 (batched DMAs), cast to bf16 on GpSimd, transposed via
  batched DMA-xbar transposes (3-D output APs -> one DMA per source tile)
  QT/KT/VT [feat, tok] = wT.T @ xT; Q/K biases added per-partition on evac
  per head a 128-col v1 block per k-chunk: V in the head's 64-partition
  half, ones in the other half (softmax denominator lands there)
  per (head, qblock, kchunk): S^T [128k, 1024q] = KT_slice.T @ QT_slice
    exp fused into the ACT PSUM evac (scale=1/8) -> P^T bf16
    attn psum [128, 1024] += v1_chunk.T @ P^T  (64 attn rows + 64 denom rows)
  normalize: evac psum, recip denom half, DMA across partition halves, mul
  out [128tok, 768] = attnT_chunk.T @ woT

Projections of feature-chunk m+1 overlap the (ScalarE-bound) attention of
heads 2m, 2m+1; PSUM budget: pj 2 + scores 4 + attn 2 = 8 banks.
DMA routing: SP HWDGE for loads/transposes, GpSimd SWDGE for stores and
the small normalize moves (the SEQ cost per dma_start is ~650ns, so DMA
count is kept low and split across the two sequencers).
"""

import numpy as np

import concourse.bass as bass
import concourse.mybir as mybir
import concourse.tile as tile
from concourse.bass_utils import run_bass_kernel_spmd

F32 = mybir.dt.float32
BF16 = mybir.dt.bfloat16
AF = mybir.ActivationFunctionType

B, S, D = 4, 2048, 768
H, HD = 12, 64
N_CORES = 8
HEADS_PER_CORE = 6          # 12 heads / 2 cores per batch
FS = HEADS_PER_CORE * HD    # 384 features per core
SCALE = 1.0 / np.sqrt(HD)   # 0.125

KT16 = S // 128             # 16 token tiles of 128
QB = 1024                   # q-block (free dim of S^T chunks)
NQB = S // QB               # 2


def split_waits(nc, cap=1):
    """walrus rejects instructions carrying >2 sync waits; the TileContext
    final drain is emitted post-lowering and can carry many. Hoist excess
    waits onto preceding same-engine NOPs (1 wait each)."""
    f = nc.m.functions[0]
    for bb in f.blocks:
        insts = list(bb.instructions)
        new = []
        changed = False
        for inst in insts:
            si = inst.sync_info
            if si is not None and si.on_wait is not None and len(si.on_wait) > cap:
                waits = list(si.on_wait)
                keep = waits[-cap:]
                extra = waits[:-cap]
                for j, w in enumerate(extra):
                    nop = mybir.InstNoOp(
                        name=f"{inst.name}-wsplit{j}",
                        engine=inst.engine,
                        ins=[], outs=[],
                        sync_info=mybir.SyncInfo(on_wait=[w], on_update=[]),
                    )
                    new.append(nop)
                    changed = True
                inst.sync_info = mybir.SyncInfo(
                    on_wait=keep, on_update=list(si.on_update or [])
                )
            new.append(inst)
        if changed:
            bb.instructions = new


def build_nc(reps=1, parts="prep,attn,out"):
    nc = bass.Bass()

    x_ext = nc.dram_tensor("xt", [D, S], F32, kind="ExternalInput")
    wq_ext = nc.dram_tensor("wqt", [D, FS], F32, kind="ExternalInput")
    wk_ext = nc.dram_tensor("wkt", [D, FS], F32, kind="ExternalInput")
    wv_ext = nc.dram_tensor("wvt", [D, FS], F32, kind="ExternalInput")
    wo_ext = nc.dram_tensor("wot", [FS, D], F32, kind="ExternalInput")
    bq_ext = nc.dram_tensor("bq", [FS], F32, kind="ExternalInput")
    bk_ext = nc.dram_tensor("bk", [FS], F32, kind="ExternalInput")
    out_ext = nc.dram_tensor("out_part", [S, D], F32, kind="ExternalOutput")

    with tile.TileContext(nc) as tc:
      for _rep in range(reps):
        with tc.tile_pool(name="persist", bufs=1) as P:
            # xT per d-chunk (tokens contiguous)
            xTc = [P.tile([128, S], BF16, name=f"xTc{j}") for j in range(6)]
            # wT block j (d-chunk) at cols j*FS
            wT = {p: P.tile([128, 6 * FS], BF16, name=f"wT{p}") for p in "qkv"}
            # woT block j (hd-chunk) at cols j*D
            woT = P.tile([128, 3 * D], BF16, name="woT")
            QT = [P.tile([128, S], BF16, name=f"QT{m}") for m in range(3)]
            KT = [P.tile([128, S], BF16, name=f"KT{m}") for m in range(3)]
            VT = [P.tile([128, S], BF16, name=f"VT{m}") for m in range(3)]
            # v1 per head: block for kchunk t at cols t*128; within a block
            # V fills cols po..po+64 (the head's attn-psum partition half),
            # the rest stays 1.0 (softmax denominator rows)
            v1 = [P.tile([128, KT16 * 128], BF16, name=f"v1h{h}")
                  for h in range(HEADS_PER_CORE)]
            attnT = {(m, q): P.tile([128, QB], BF16, name=f"attnT{m}_{q}")
                     for m in range(3) for q in range(NQB)}
            qb_sb = P.tile([128, 3], F32, name="qb_sb")
            kb_sb = P.tile([128, 3], F32, name="kb_sb")

            nc.sync.dma_start(qb_sb[:], bq_ext.rearrange("(j p) -> p j", p=128))
            nc.sync.dma_start(kb_sb[:], bk_ext.rearrange("(j p) -> p j", p=128))
            warm = P.tile([128, 1], F32, name="warm")
            nc.vector.memset(warm[:], 0.0)
            nc.scalar.activation(warm[:], warm[:], AF.Exp)

            v13 = [t[:].rearrange("p (t q) -> p t q", t=KT16) for t in v1]

            with (
                tc.tile_pool(name="ld", bufs=2) as L,
                tc.tile_pool(name="cst", bufs=2) as C,
                tc.tile_pool(name="pj", bufs=2, space="PSUM") as PJ,
                tc.tile_pool(name="sp", bufs=2, space="PSUM") as SP,
                tc.tile_pool(name="atp", bufs=1, space="PSUM") as AT,
                tc.tile_pool(name="nw", bufs=2) as NW,
                tc.tile_pool(name="pw", bufs=3) as PW,
            ):
                # ---- phase A: inputs arrive pre-transposed; load + cast --
                for wi, (p, ext) in enumerate(
                        (("q", wq_ext), ("k", wk_ext), ("v", wv_ext))):
                    wt = L.tile([128, 6 * FS], F32, tag="wld", name="wload")
                    nc.scalar.dma_start(
                        wt[:].rearrange("p (j f) -> p j f", j=6),
                        ext[:].rearrange("(j p) f -> p j f", p=128))
                    ceng = nc.gpsimd if wi < 2 else nc.vector
                    ceng.tensor_copy(wT[p][:], wt[:])

                wt = L.tile([128, 3 * D], F32, tag="wld", name="woload")
                nc.scalar.dma_start(
                    wt[:].rearrange("p (j f) -> p j f", j=3),
                    wo_ext[:].rearrange("(j p) f -> p j f", p=128))
                nc.gpsimd.tensor_copy(woT[:], wt[:])

                for j in range(6):
                    xt = L.tile([128, S], F32, tag="xld", name="xload")
                    nc.sync.dma_start(xt[:], x_ext[j * 128:(j + 1) * 128, :])
                    ceng = nc.gpsimd if j % 2 else nc.vector
                    ceng.tensor_copy(xTc[j][:], xt[:])

                # ---- projections m interleaved with attention 2m, 2m+1 ---
                for m in range(3):
                    projs = (("q", QT, qb_sb), ("k", KT, kb_sb))
                    if m != 0 or "attn" not in parts:
                        projs = projs + (("v", VT, None),)
                    for p, dsts, bias in projs:
                        for s4 in range(4):
                            ps = PJ.tile([128, 512], F32, tag="pj", name="pj")
                            for kc in range(6):
                                nc.tensor.matmul(
                                    ps[:],
                                    wT[p][:, kc * FS + m * 128:
                                          kc * FS + (m + 1) * 128],
                                    xTc[kc][:, s4 * 512:(s4 + 1) * 512],
                                    start=(kc == 0), stop=(kc == 5),
                                )
                            dst = dsts[m][:, s4 * 512:(s4 + 1) * 512]
                            if bias is not None:
                                nc.vector.tensor_scalar_add(
                                    dst, ps[:], bias[:, m:m + 1])
                            else:
                                nc.vector.tensor_copy(dst, ps[:])

                    pre = []
                    if m == 0 and "attn" in parts:
                        # start the ScalarE exp stream before the V projection:
                        # head 0 / qb 0 scores+exp for kc<8, AV deferred
                        for kc in range(8):
                            sps = SP.tile([128, QB], F32, tag="s", name="s")
                            for j in range(QB // 512):
                                nc.tensor.matmul(
                                    sps[:, j * 512:(j + 1) * 512],
                                    KT[0][0:64, kc * 128:(kc + 1) * 128],
                                    QT[0][0:64, j * 512:(j + 1) * 512],
                                    start=True, stop=True,
                                )
                            pt = PW.tile([128, QB], BF16, tag="p", name="p", bufs=11)
                            nc.scalar.activation(pt[:], sps[:], AF.Exp, scale=SCALE)
                            pre.append(pt)
                        for p, dsts, bias in (("v", VT, None),):
                            for s4 in range(4):
                                ps = PJ.tile([128, 512], F32, tag="pj", name="pj")
                                for kc in range(6):
                                    nc.tensor.matmul(
                                        ps[:],
                                        wT[p][:, kc * FS + m * 128:
                                              kc * FS + (m + 1) * 128],
                                        xTc[kc][:, s4 * 512:(s4 + 1) * 512],
                                        start=(kc == 0), stop=(kc == 5),
                                    )
                                nc.vector.tensor_copy(
                                    dsts[m][:, s4 * 512:(s4 + 1) * 512], ps[:])

                    for h in (2 * m, 2 * m + 1):
                        po = (h % 2) * 64
                        (nc.vector if h % 2 else nc.gpsimd).memset(v1[h][:], 1.0)
                        nc.sync.dma_start_transpose(
                            v13[h][:, :, po:po + 64],
                            VT[m][po:po + 64, :])

                    if "attn" not in parts:
                        continue
                    for h in (2 * m, 2 * m + 1):
                        po = (h % 2) * 64
                        pd = 64 - po
                        for qb in range(NQB):
                            at = AT.tile([128, QB], F32, tag="at", name="at")
                            start_kc = 0
                            if m == 0 and h == 0 and qb == 0 and pre:
                                for kc, pt in enumerate(pre):
                                    for j in range(QB // 512):
                                        nc.tensor.matmul(
                                            at[:, j * 512:(j + 1) * 512],
                                            v1[h][:, kc * 128:(kc + 1) * 128],
                                            pt[:, j * 512:(j + 1) * 512],
                                            start=(kc == 0), stop=False,
                                        )
                                start_kc = len(pre)
                            for kc in range(start_kc, KT16):
                                sps = SP.tile([128, QB], F32, tag="s", name="s")
                                for j in range(QB // 512):
                                    nc.tensor.matmul(
                                        sps[:, j * 512:(j + 1) * 512],
                                        KT[m][po:po + 64,
                                              kc * 128:(kc + 1) * 128],
                                        QT[m][po:po + 64,
                                              qb * QB + j * 512:
                                              qb * QB + (j + 1) * 512],
                                        start=True, stop=True,
                                    )
                                pt = PW.tile([128, QB], BF16, tag="p", name="p", bufs=11)
                                nc.scalar.activation(
                                    pt[:], sps[:], AF.Exp, scale=SCALE)
                                for j in range(QB // 512):
                                    nc.tensor.matmul(
                                        at[:, j * 512:(j + 1) * 512],
                                        v1[h][:, kc * 128:(kc + 1) * 128],
                                        pt[:, j * 512:(j + 1) * 512],
                                        start=(kc == 0), stop=(kc == KT16 - 1),
                                    )
                            # evac attn psum quickly (frees the psum slot),
                            # then normalize: recip on the denominator half,
                            # DMA it across partition halves, multiply.
                            ats = NW.tile([128, QB], F32, tag="ats", name="ats")
                            nc.vector.tensor_copy(ats[:], at[:])
                            rc = NW.tile([128, QB], F32, tag="rc", name="rc")
                            nc.vector.reciprocal(
                                rc[pd:pd + 64, :], ats[pd:pd + 64, :])
                            nc.gpsimd.dma_start(
                                rc[po:po + 64, :], rc[pd:pd + 64, :])
                            nc.vector.tensor_mul(
                                attnT[(m, qb)][po:po + 64, :],
                                ats[po:po + 64, :], rc[po:po + 64, :])

                # ---- output projection (PSUM from the pj pool, so it
                # overlaps the tail of attention) ----------------------
                if "out" not in parts:
                    continue
                for t in range(KT16):
                    ot = PW.tile([128, D], F32, tag="ot", name="ot")
                    for (lo, hi) in ((0, 512), (512, 768)):
                        ps = PJ.tile([128, hi - lo], F32, tag="pj", name="opj")
                        for kc in range(3):
                            nc.tensor.matmul(
                                ps[:],
                                attnT[(kc, t // 8)][:, (t % 8) * 128:
                                                    (t % 8 + 1) * 128],
                                woT[:, kc * D + lo:kc * D + hi],
                                start=(kc == 0), stop=(kc == 2),
                            )
                        oeng = nc.vector if t % 2 else nc.scalar
                        if oeng is nc.scalar:
                            nc.scalar.activation(ot[:, lo:hi], ps[:], AF.Copy)
                        else:
                            nc.vector.tensor_copy(ot[:, lo:hi], ps[:])
                    nc.gpsimd.dma_start(out_ext[t * 128:(t + 1) * 128, :], ot[:])

    split_waits(nc)
    return nc


_NC_CACHE = None


def _get_nc():
    global _NC_CACHE
    if _NC_CACHE is None:
        _NC_CACHE = build_nc()
    return _NC_CACHE


def make_in_maps(x, QW_w, QW_b, KW_w, KW_b, VW_w, VW_b, OW_w, OW_b):
    f32 = lambda a: np.ascontiguousarray(np.asarray(a), dtype=np.float32)
    in_maps = []
    for c in range(N_CORES):
        b, hh = c // 2, c % 2
        sl = slice(hh * FS, (hh + 1) * FS)
        in_maps.append({
            "xt": f32(np.asarray(x[b]).T),
            "wqt": f32(np.asarray(QW_w)[sl, :].T),
            "wkt": f32(np.asarray(KW_w)[sl, :].T),
            "wvt": f32(np.asarray(VW_w)[sl, :].T),
            "wot": f32(np.asarray(OW_w)[:, sl].T),
            "bq": f32(QW_b[sl]),
            "bk": f32(KW_b[sl]),
        })
    return in_maps


def kernel(x, QW_w, QW_b, KW_w, KW_b, VW_w, VW_b, OW_w, OW_b):
    nc = _get_nc()
    in_maps = make_in_maps(x, QW_w, QW_b, KW_w, KW_b, VW_w, VW_b, OW_w, OW_b)
    res = run_bass_kernel_spmd(nc, in_maps, list(range(N_CORES)))

    out = np.zeros((B, S, D), dtype=np.float32)
    OW_w = np.asarray(OW_w, dtype=np.float32)
    OW_b = np.asarray(OW_b, dtype=np.float32)
    VW_b = np.asarray(VW_b, dtype=np.float32)
    for c in range(N_CORES):
        b = c // 2
        out[b] += res.results[c]["out_part"]
    for b in range(B):
        # OW bias + V-bias routed through the output projection
        out[b] += OW_b + OW_w @ VW_b
    return out

